# revision 5
# baseline (speedup 1.0000x reference)
"""GCN message-passing kernel for Trainium2, distributed over 8 NeuronCores.

Strategy (dst-sharded, per the sharding hint):
  - Nodes and their incoming edges are partitioned by dst id: core k owns dst
    rows [k*12500, (k+1)*12500).  Each core receives its own copy of the
    source-feature table, so the x[src] gather is local and no collectives
    are needed.
  - Per core the source table is renumbered by edge frequency (descending) so
    gather indices fit the int16 index format of the dma_gather unit in at
    most 3 buckets of 32768 rows.  The table is stored in fp16 with rows
    padded to 128 elements (256B) to satisfy the gather unit's 256B element
    granularity; the fp16 pipeline runs the PE at 1 cycle/row (vs 4 for
    fp32) and the DVE one-hot build at 2 elem/cycle.
  - Edges are grouped by (dst-tile of 512 slots, src bucket) into 128-edge
    chunks; trailing chunk padding uses index -1, which the gather unit skips
    entirely (no HBM descriptor), so gather traffic is within ~3% of the true
    edge count.  Gathers round-robin over 4 SWDGE queues.
  - Per chunk: dma_gather delivers G [128 edges, 128 (64 used)] fp16; a
    one-hot S [128 edges, 512 slots] fp16 is built on the vector engine via
    is_equal(dst_slot, iota); TensorE accumulates aggT += G.T @ S in PSUM.
  - Per tile: h = [aggT; 1s].T @ [W; b] in 128-slot sub-matmuls (bias folded
    in via a ones row appended to aggT and b appended to W), scalar-engine
    PSUM->SBUF copies, then DMA to the contiguous output shard.
"""
import sys

sys.path.insert(0, "/opt/trn_rl_repo")

import numpy as np

N_NODES = 100000
N_CORES = 8
SLOTS = 512


P = 128
BUCKET = 32768
D = 64
ROW = 128  # fp16 row stride of the gather table (64 used + 64 pad = 256B)


def prepare3(edge_src, edge_dst, n_nodes, n_cores, slots=512, max_call=8):
    es = np.asarray(edge_src).astype(np.int64)
    ed = np.asarray(edge_dst).astype(np.int64)
    npc = n_nodes // n_cores
    tpc = (npc + slots - 1) // slots

    core = ed // npc
    dst_local = ed - core * npc
    tile = dst_local // slots
    slot = (dst_local % slots).astype(np.float32)

    ren = np.empty((n_cores,), dtype=object)
    n_u = np.zeros(n_cores, dtype=np.int64)
    ridx_all = np.zeros(len(es), dtype=np.int64)
    for k in range(n_cores):
        m = core == k
        uniq, inv, cnt = np.unique(es[m], return_inverse=True, return_counts=True)
        order = np.argsort(-cnt, kind="stable")
        rank_of = np.empty(len(uniq), dtype=np.int64)
        rank_of[order] = np.arange(len(uniq))
        ridx_all[m] = rank_of[inv]
        ren[k] = uniq[order]
        n_u[k] = len(uniq)
    x_rows = int(((n_u.max() + P - 1) // P) * P)
    n_buckets = int((x_rows + BUCKET - 1) // BUCKET)
    assert n_buckets <= 3

    bucket = ridx_all // BUCKET
    rel = (ridx_all - bucket * BUCKET).astype(np.int32)

    gid = (core * tpc + tile) * n_buckets + bucket
    n_g = n_cores * tpc * n_buckets
    counts = np.bincount(gid, minlength=n_g).reshape(n_cores, tpc, n_buckets)
    gmax = counts.max(axis=0)                   # real+filler count per (t,b)
    c_tb = (gmax + P - 1) // P                  # chunks per (t,b)
    n_ch = int(c_tb.sum())

    # chunk-column offsets; calls split within each (t, b) group
    off_tb = np.zeros((tpc, n_buckets), dtype=np.int64)
    calls = []  # (chunk_off, n_chunks, bucket, n_real)
    pos = 0
    for t in range(tpc):
        for b in range(n_buckets):
            off_tb[t, b] = pos
            c = int(c_tb[t, b])
            rem = int(gmax[t, b])
            o = pos
            while c > 0:
                take = min(c, max_call)
                nreal = min(rem, take * P)
                calls.append((o, take, b, nreal))
                o += take
                c -= take
                rem -= nreal
            pos += int(c_tb[t, b])
    assert pos == n_ch

    order = np.argsort(gid, kind="stable")
    g_sorted = gid[order]
    gstart = np.concatenate([[0], np.cumsum(np.bincount(g_sorted, minlength=n_g))])[:-1]
    rank = np.arange(len(es)) - gstart[g_sorted]
    core_o = g_sorted // (tpc * n_buckets)
    t_o = (g_sorted // n_buckets) % tpc
    b_o = g_sorted % n_buckets
    colpos = off_tb[t_o, b_o] + rank // P
    ppos = rank % P

    # defaults: -1 (skipped by gather).  Real edges and 0-filler (to gmax)
    # overwrite below.
    idx = np.full((n_cores, P, n_ch), -1, dtype=np.int16)
    dcol = np.full((n_cores, P, n_ch), -1.0, dtype=np.float32)
    idx[core_o, ppos, colpos] = rel[order].astype(np.int16)
    dcol[core_o, ppos, colpos] = slot[order]
    # 0-filler region: positions [count(core,t,b), gmax(t,b)) must be >= 0 so
    # n_real is core-independent; point them at row 0 of the bucket.
    for k in range(n_cores):
        for t in range(tpc):
            for b in range(n_buckets):
                lo = int(counts[k, t, b])
                hi = int(gmax[t, b])
                if lo < hi:
                    o = off_tb[t, b]
                    pp = np.arange(lo, hi)
                    idx[k, pp % P, o + pp // P] = 0
    idx_flat = idx.transpose(0, 2, 1).reshape(n_cores, n_ch * P)
    w = idx_flat.reshape(n_cores, n_ch * P // 16, 16).transpose(0, 2, 1)
    idx_wrap = np.ascontiguousarray(np.tile(w, (1, 8, 1)))

    meta = {
        "c_tb": c_tb, "off_tb": off_tb, "calls": calls, "n_ch": n_ch,
        "tpc": tpc, "npc": npc, "x_rows": x_rows, "n_buckets": n_buckets,
        "slots": slots,
    }
    return idx_wrap, dcol, ren, meta


def build3(meta, n_cores, max_call_chunks=8, repeat=1):
    import concourse.bass as bass
    import concourse.bacc as bacc
    import concourse.mybir as mybir
    import concourse.tile as tile

    f32 = mybir.dt.float32
    f16 = mybir.dt.float16
    i16 = mybir.dt.int16
    slots = meta["slots"]
    c_tb = meta["c_tb"]
    off_tb = meta["off_tb"]
    calls = meta["calls"]
    n_ch = meta["n_ch"]
    tpc = meta["tpc"]
    npc = meta["npc"]
    x_rows = meta["x_rows"]
    n_buckets = meta["n_buckets"]

    nc = bacc.Bacc("TRN2", target_bir_lowering=False, debug=False,
                   num_devices=n_cores)

    x_d = nc.dram_tensor("x", [x_rows, ROW], f16, kind="ExternalInput")
    idx_d = nc.dram_tensor("idx", [P, n_ch * 8], i16, kind="ExternalInput")
    dcol_d = nc.dram_tensor("dcol", [P, n_ch], f32, kind="ExternalInput")
    iota_d = nc.dram_tensor("iota", [P, slots], f16, kind="ExternalInput")
    w_d = nc.dram_tensor("W", [D + 1, D], f32, kind="ExternalInput")
    out_d = nc.dram_tensor("out", [npc, D], f32, kind="ExternalOutput")

    with tile.TileContext(nc) as tc:
        with (
            tc.tile_pool(name="const", bufs=1) as cp,
            tc.tile_pool(name="g", bufs=16) as g_pool,
            tc.tile_pool(name="s", bufs=12) as s_pool,
            tc.tile_pool(name="ag", bufs=3) as ag_pool,
            tc.tile_pool(name="h", bufs=4) as h_pool,
            tc.tile_pool(name="psA", bufs=4, space="PSUM") as psA,
            tc.tile_pool(name="psH", bufs=4, space="PSUM") as psH,
        ):
            idx_sb = cp.tile([P, n_ch * 8], i16)
            dcol_sb = cp.tile([P, n_ch], f32)
            iota_sb = cp.tile([P, slots], f16)
            w_sb = cp.tile([D + 1, D], f32)

            nc.sync.dma_start(out=idx_sb[:], in_=idx_d[:])
            nc.sync.dma_start(out=dcol_sb[:], in_=dcol_d[:])
            nc.sync.dma_start(out=iota_sb[:], in_=iota_d[:])
            nc.sync.dma_start(out=w_sb[:], in_=w_d[:])

            for _rep in range(repeat):
                chunk_home = {}
                for ci, (o, ncall, b, nreal) in enumerate(calls):
                    g = g_pool.tile([P, ncall * ROW], f16, tag="g")
                    if nreal < ncall * P:
                        nc.vector.memzero(g[:])
                    base = b * BUCKET
                    hi = min(base + BUCKET, x_rows)
                    nc.gpsimd.dma_gather(
                        out_ap=g[:].rearrange("p (k e) -> p k e", e=ROW),
                        in_ap=x_d[base:hi, :],
                        idxs_ap=idx_sb[:, o * 8 : (o + ncall) * 8],
                        num_idxs=ncall * P,
                        num_idxs_reg=nreal,
                        elem_size=ROW,
                    )
                    for j in range(ncall):
                        chunk_home[o + j] = (g, j)

                for t in range(tpc):
                    cols = []
                    for b in range(n_buckets):
                        for j in range(int(c_tb[t, b])):
                            cols.append(int(off_tb[t, b]) + j)
                    tile_slots = min(slots, npc - t * slots)
                    nsub = (tile_slots + P - 1) // P
                    ags = ag_pool.tile([D + 1, slots], f32)
                    if cols:
                        agp = psA.tile([D, slots], f32)
                        for i, c in enumerate(cols):
                            g, j = chunk_home[c]
                            s = s_pool.tile([P, slots], f16, tag="s")
                            nc.vector.tensor_scalar(
                                out=s[:],
                                in0=iota_sb[:],
                                scalar1=dcol_sb[:, c : c + 1],
                                scalar2=None,
                                op0=mybir.AluOpType.is_equal,
                            )
                            nc.tensor.matmul(
                                out=agp[:],
                                lhsT=g[:, j * ROW : j * ROW + D],
                                rhs=s[:],
                                start=(i == 0),
                                stop=(i == len(cols) - 1),
                            )
                        nc.scalar.copy(out=ags[:D, :], in_=agp[:])
                    else:
                        nc.vector.memzero(ags[:D, :])
                    nc.gpsimd.memset(ags[D : D + 1, :], 1.0)
                    for sub in range(nsub):
                        rows = min(P, tile_slots - sub * P)
                        hp = psH.tile([P, D], f32)
                        nc.tensor.matmul(
                            out=hp[:],
                            lhsT=ags[:, sub * P : sub * P + P],
                            rhs=w_sb[:],
                            start=True,
                            stop=True,
                        )
                        hs = h_pool.tile([P, D], f32)
                        nc.scalar.copy(out=hs[:], in_=hp[:])
                        r0 = t * slots + sub * P
                        nc.sync.dma_start(
                            out=out_d[r0 : r0 + rows, :], in_=hs[:rows, :]
                        )

    nc.compile()
    return nc


def make_x_cores(x, ren, x_rows):
    out = []
    for k in range(len(ren)):
        xk = np.zeros((x_rows, ROW), dtype=np.float16)
        xk[: len(ren[k]), :D] = x[ren[k]]
        out.append(xk)
    return out


def kernel(x, edge_src, edge_dst, W, b):
    from concourse.bass_utils import run_bass_kernel_spmd

    idx_wrap, dcol, ren, meta = prepare3(
        edge_src, edge_dst, N_NODES, N_CORES, slots=SLOTS
    )
    nc = build3(meta, N_CORES)
    x_cores = make_x_cores(np.asarray(x, dtype=np.float32), ren, meta["x_rows"])
    iota = np.ascontiguousarray(
        np.arange(SLOTS, dtype=np.float16)[None, :].repeat(P, axis=0)
    )
    w_ext = np.concatenate(
        [np.asarray(W, np.float32), np.asarray(b, np.float32).reshape(1, D)], axis=0
    )
    maps = []
    for k in range(N_CORES):
        maps.append(
            {
                "x": x_cores[k],
                "idx": np.ascontiguousarray(idx_wrap[k]),
                "dcol": np.ascontiguousarray(dcol[k]),
                "iota": iota,
                "W": np.ascontiguousarray(w_ext),
            }
        )
    res = run_bass_kernel_spmd(nc, maps, list(range(N_CORES)))
    out = np.concatenate([res.results[k]["out"] for k in range(N_CORES)], axis=0)
    return out.astype(np.float32)
